# revision 2
# baseline (speedup 1.0000x reference)
"""Two-layer GCN block on 8 Trainium2 NeuronCores — full-featured variant.

Flags: BATCH_ST (batched selection build), NQ (SWDGE queues), DENSE_SELF
(self-loops handled densely), OUT_BF16, CHUNK_AG (chunked AllGather overlapped
with layer-1 tail; layer-2 gathers read per-chunk tables).

All gather padding uses index 0 (NEVER -1: trimmed calls break the DMA
completion semaphore contract and hang the device).
"""

import os
import sys

if "/opt/trn_rl_repo" not in sys.path:
    sys.path.insert(0, "/opt/trn_rl_repo")
os.environ.setdefault("NEURON_SCRATCHPAD_PAGE_SIZE", "512")

from dataclasses import dataclass

import ml_dtypes
import numpy as np

P = 128

BATCH_ST = True
NQ = 4
DENSE_SELF = True
OUT_BF16 = True
CHUNK_AG = True
MERGE_GATHER = False
SINGLE_PACKET = False


@dataclass(frozen=True)
class Cfg:
    n: int
    n_cores: int = 8
    scw: int = 7
    blk: int = 32768
    cw: int = 25
    nq: int = 2
    batch_st: bool = False
    dense_self: bool = False
    out_bf16: bool = False
    chunk_ag: bool = False
    merge_gather: bool = False
    single_packet: bool = False

    @property
    def npc(self):
        return self.n // self.n_cores

    @property
    def wpc(self):
        return -(-self.npc // P)

    @property
    def ppc(self):
        return self.wpc * P

    @property
    def npad(self):
        return self.ppc * self.n_cores

    @property
    def nwin(self):
        return self.wpc * self.n_cores

    @property
    def nsc(self):
        assert self.wpc % self.scw == 0
        return self.wpc // self.scw

    @property
    def nb(self):
        return -(-self.npad // self.blk)

    @property
    def nchunk(self):
        return -(-self.wpc // self.cw)

    @property
    def chunk_w(self):
        return [min(self.cw, self.wpc - k * self.cw) for k in range(self.nchunk)]


def make_cfg():
    return Cfg(
        n=100000,
        nq=NQ,
        batch_st=BATCH_ST,
        dense_self=DENSE_SELF,
        out_bf16=OUT_BF16,
        chunk_ag=CHUNK_AG,
        merge_gather=MERGE_GATHER,
        single_packet=SINGLE_PACKET,
    )


# ----------------------------------------------------------------------------
# Host-side preprocessing
# ----------------------------------------------------------------------------


def _mk_sched(cfg: Cfg, core, locw, bucket, rel, lidx, nbuckets):
    ncores, wpc, nsc, scw = cfg.n_cores, cfg.wpc, cfg.nsc, cfg.scw
    key = (core * wpc + locw) * nbuckets + bucket
    counts = np.bincount(key, minlength=ncores * wpc * nbuckets).reshape(
        ncores, wpc, nbuckets
    )
    G = (-(-counts // P)).max(axis=0)
    gtot = int(G.sum())

    goff = np.zeros((wpc, nbuckets), dtype=np.int64)
    np.cumsum(G.ravel()[:-1], out=goff.ravel()[1:])

    ni = np.zeros((nsc, nbuckets), dtype=np.int64)
    for s in range(nsc):
        ni[s] = G[s * scw : (s + 1) * scw].sum(axis=0) * P
    icoff = np.zeros(nsc * nbuckets, dtype=np.int64)
    np.cumsum(ni.ravel()[:-1] // 16, out=icoff[1:])
    icoff = icoff.reshape(nsc, nbuckets)
    icols = int(ni.sum() // 16)

    boff = np.zeros((wpc, nbuckets), dtype=np.int64)
    for s in range(nsc):
        js = slice(s * scw, (s + 1) * scw)
        boff[js] = np.cumsum(
            np.vstack([np.zeros(nbuckets, np.int64), G[js][:-1] * P]), axis=0
        )

    rd_all = np.full((ncores, P, gtot), -1.0, dtype=np.float32)
    idx_all = np.zeros((ncores, P, icols), dtype=np.int16)

    order = np.lexsort((lidx, bucket, locw, core))
    srt_core = core[order]
    srt_key = (locw * nbuckets + bucket)[order]
    srt_rel = rel[order]
    srt_lidx = lidx[order]

    core_bounds = np.searchsorted(srt_core, np.arange(ncores + 1))
    for c in range(ncores):
        lo, hi = core_bounds[c], core_bounds[c + 1]
        k = srt_key[lo:hi]
        r = srt_rel[lo:hi]
        ls = srt_lidx[lo:hi]
        bucket_start = np.searchsorted(k, np.arange(wpc * nbuckets))
        q = np.arange(k.size) - bucket_start[k]
        w = k // nbuckets
        b = k % nbuckets
        col = goff[w, b] + q // P
        rd_all[c, q % P, col] = r
        i = boff[w, b] + q
        s = w // scw
        idx_all[c, (i % 16), icoff[s, b] + i // 16] = ls
    idx_all = np.tile(idx_all[:, :16, :], (1, 8, 1))
    rd_all = rd_all.astype(ml_dtypes.bfloat16)

    return {
        "G": G,
        "ni": ni,
        "icoff": icoff,
        "gtot": gtot,
        "icols": icols,
        "rd_all": rd_all,
        "idx_all": idx_all,
    }


def _preprocess(cfg: Cfg, edge_index: np.ndarray):
    n = cfg.n
    if cfg.dense_self:
        src = edge_index[0].astype(np.int64)
        dst = edge_index[1].astype(np.int64)
        deg = (np.bincount(dst, minlength=n) + 1).astype(np.float32)
    else:
        src = np.concatenate([edge_index[0], np.arange(n, dtype=np.int64)])
        dst = np.concatenate([edge_index[1], np.arange(n, dtype=np.int64)])
        deg = np.bincount(dst, minlength=n).astype(np.float32)

    def pad_id(v):
        c = v // cfg.npc
        return c * cfg.ppc + (v - c * cfg.npc)

    srcp = pad_id(src)
    dstp = pad_id(dst)
    core = dst // cfg.npc
    locw = (dstp % cfg.ppc) // P
    rel = (dstp % P).astype(np.float32)

    b1 = srcp // cfg.blk
    l1 = (srcp - b1 * cfg.blk).astype(np.int16)
    s1 = _mk_sched(cfg, core, locw, b1, rel, l1, cfg.nb)

    out = {"s1": s1, "pad_id": pad_id}
    if cfg.chunk_ag:
        csrc = srcp // cfg.ppc
        t = (srcp % cfg.ppc) // P
        p = srcp % P
        k2 = np.minimum(t // cfg.cw, cfg.nchunk - 1)
        cw_arr = np.asarray(cfg.chunk_w, dtype=np.int64)
        t0 = np.concatenate([[0], np.cumsum(cw_arr)[:-1]])
        l2v = csrc * (cw_arr[k2] * P) + (t - t0[k2]) * P + p
        assert l2v.max() < 32768
        s2 = _mk_sched(cfg, core, locw, k2, rel, l2v.astype(np.int16), cfg.nchunk)
        out["s2"] = s2
    else:
        out["s2"] = s1

    degp = np.ones(cfg.npad, dtype=np.float32)
    degp[pad_id(np.arange(n))] = deg
    out["deg_all"] = degp.reshape(cfg.nwin, P).T.copy()
    out["degp"] = degp
    return out


# ----------------------------------------------------------------------------
# Device program
# ----------------------------------------------------------------------------


def _build(cfg: Cfg, s1, s2, no_collective: bool = False, reps: int = 1):
    import concourse.bacc as bacc
    import concourse.mybir as mybir
    import concourse.tile as tile

    f32 = mybir.dt.float32
    bf16 = mybir.dt.bfloat16
    i16 = mybir.dt.int16
    EQ = mybir.AluOpType.is_equal
    MUL = mybir.AluOpType.mult
    ADD = mybir.AluOpType.add
    ACT_COPY = mybir.ActivationFunctionType.Copy
    ACT_RELU = mybir.ActivationFunctionType.Relu

    nc = bacc.Bacc(
        "TRN2", target_bir_lowering=False, debug=False, num_swdge_queues=cfg.nq
    )

    out_dt = bf16 if cfg.out_bf16 else f32

    xs = nc.declare_dram_parameter("xs", [cfg.npad, P], bf16, isOutput=False)
    if cfg.dense_self:
        xso = nc.declare_dram_parameter("xso", [cfg.ppc, P], bf16, isOutput=False)
    w1 = nc.declare_dram_parameter("w1", [P, P], bf16, isOutput=False)
    w2 = nc.declare_dram_parameter("w2", [P, P], bf16, isOutput=False)
    b2r = nc.declare_dram_parameter("b2r", [P, P], f32, isOutput=False)
    iota = nc.declare_dram_parameter("iota", [P, P], bf16, isOutput=False)
    deg_own = nc.declare_dram_parameter("deg_own", [P, cfg.wpc], f32, isOutput=False)
    idx1 = nc.declare_dram_parameter("idx1", [P, s1["icols"]], i16, isOutput=False)
    rd1 = nc.declare_dram_parameter("rd1", [P, s1["gtot"]], bf16, isOutput=False)
    if cfg.chunk_ag:
        idx2 = nc.declare_dram_parameter("idx2", [P, s2["icols"]], i16, isOutput=False)
        rd2 = nc.declare_dram_parameter("rd2", [P, s2["gtot"]], bf16, isOutput=False)
    else:
        idx2, rd2 = idx1, rd1
    out = nc.declare_dram_parameter("out", [cfg.ppc, P], out_dt, isOutput=True)

    cws = cfg.chunk_w
    if cfg.chunk_ag:
        ts2own = [
            nc.dram_tensor(f"ts2own{k}", [cws[k] * P, P], bf16)
            for k in range(cfg.nchunk)
        ]
        ts2f = [
            nc.dram_tensor(
                f"ts2f{k}", [cfg.n_cores * cws[k] * P, P], bf16, addr_space="Shared"
            )
            for k in range(cfg.nchunk)
        ]
        chunk_of_w, row_of_w = [], []
        for k, wk in enumerate(cws):
            for i in range(wk):
                chunk_of_w.append(k)
                row_of_w.append(i)
        chunk_ends = set(int(x) for x in np.cumsum(cws) - 1)
    else:
        ts2s = nc.dram_tensor("ts2s", [cfg.ppc, P], bf16)
        ts2f0 = nc.dram_tensor("ts2f", [cfg.npad, P], bf16, addr_space="Shared")
        ts2s_r = ts2s.ap().rearrange("(t p) f -> p t f", p=P)

    out_r = out.ap().rearrange("(t p) f -> p t f", p=P)
    n_blk_rows = [min(cfg.blk, cfg.npad - b * cfg.blk) for b in range(cfg.nb)]

    qctr = [0]

    with tile.TileContext(nc) as tc:
        cpool = tc.tile_pool(name="const", bufs=1)
        cp = cpool.__enter__()
        w1_t = cp.tile([P, P], bf16)
        nc.sync.dma_start(w1_t[:], w1[:, :])
        w2_t = cp.tile([P, P], bf16)
        nc.sync.dma_start(w2_t[:], w2[:, :])
        b2_t = cp.tile([P, P], f32)
        nc.sync.dma_start(b2_t[:], b2r[:, :])
        iota_t = cp.tile([P, P], bf16)
        nc.sync.dma_start(iota_t[:], iota[:, :])

        dego_t = cp.tile([P, cfg.wpc], f32)
        nc.sync.dma_start(dego_t[:], deg_own[:, :])
        rcpo_t = cp.tile([P, cfg.wpc], f32)
        nc.vector.reciprocal(rcpo_t[:], dego_t[:])
        dinv_own = cp.tile([P, cfg.wpc], f32)
        nc.scalar.activation(dinv_own[:], rcpo_t[:], mybir.ActivationFunctionType.Sqrt)
        dinv2_own = cp.tile([P, cfg.wpc], f32)
        nc.vector.tensor_tensor(
            out=dinv2_own[:], in0=dinv_own[:], in1=dinv_own[:], op=MUL
        )

        def aggregate(layer1: bool, rep: int = 0):
            sch = s1 if layer1 else s2
            G, ni, icoff = sch["G"], sch["ni"], sch["icoff"]
            nbuck = cfg.nb if layer1 or not cfg.chunk_ag else cfg.nchunk
            rd_t = rd1 if layer1 else rd2
            idx_t = idx1 if layer1 else idx2
            sfx = ("1" if layer1 else "2") + (f"r{rep}" if rep else "")
            with (
                tc.tile_pool(name=f"pg_idx{sfx}", bufs=2) as p_idx,
                tc.tile_pool(name=f"pg_msg{sfx}", bufs=2) as p_msg,
                tc.tile_pool(name=f"pg_rd{sfx}", bufs=2) as p_rd,
                tc.tile_pool(name=f"pg_dn{sfx}", bufs=2) as p_dn,
                tc.tile_pool(name=f"pg_s{sfx}", bufs=3) as p_s,
                tc.tile_pool(name=f"pg_eo{sfx}", bufs=3) as p_eo,
                tc.tile_pool(name=f"pg_ps{sfx}", bufs=2, space="PSUM") as p_ps,
                tc.tile_pool(name=f"pg_ps{sfx}b", bufs=2, space="PSUM") as p_ps2,
            ):
                rdcol = 0
                for s in range(cfg.nsc):
                    js = slice(s * cfg.scw, (s + 1) * cfg.scw)
                    j0 = s * cfg.scw
                    sc_cols = int(G[js].sum())
                    sc_rd0 = rdcol
                    rdt = p_rd.tile([P, max(1, sc_cols)], bf16, tag="rdt")
                    if sc_cols:
                        nc.sync.dma_start(
                            rdt[:, :sc_cols], rd_t[:, rdcol : rdcol + sc_cols]
                        )

                    if cfg.dense_self:
                        dn = p_dn.tile([P, cfg.scw * P], bf16, tag="dn")
                        if layer1:
                            r0 = j0 * P
                            nc.sync.dma_start(
                                dn[:], xso[r0 : r0 + cfg.scw * P, :], transpose=True
                            )
                        elif cfg.chunk_ag:
                            for jl in range(cfg.scw):
                                j = j0 + jl
                                k = chunk_of_w[j]
                                nc.sync.dma_start(
                                    dn[:, jl * P : (jl + 1) * P],
                                    ts2own[k]
                                    .ap()
                                    .rearrange("(t p) f -> p t f", p=P)[
                                        :, row_of_w[j], :
                                    ],
                                )
                        else:
                            nc.sync.dma_start(
                                dn[:].rearrange("p (t f) -> p t f", f=P),
                                ts2s_r[:, js, :],
                            )

                    mg = []
                    for b in range(nbuck):
                        nib = int(ni[s, b])
                        mx = int(ni[:, b].max())
                        mt = p_msg.tile([P, max(P, mx)], bf16, tag=f"mg{b}")
                        if nib:
                            it = p_idx.tile([P, max(1, mx // 16)], i16, tag=f"it{b}")
                            nc.sync.dma_start(
                                it[:, : nib // 16],
                                idx_t[:, icoff[s, b] : icoff[s, b] + nib // 16],
                            )
                            if layer1:
                                table = xs[b * cfg.blk : b * cfg.blk + n_blk_rows[b], :]
                            elif cfg.chunk_ag:
                                table = ts2f[b].ap()[:, :]
                            else:
                                table = ts2f0.ap()[
                                    b * cfg.blk : b * cfg.blk + n_blk_rows[b], :
                                ]
                            if cfg.merge_gather:
                                # padding rows gather row 0 (masked by rd=-1),
                                # so one call covers the whole (s,b) bucket
                                nc.gpsimd.dma_gather(
                                    out_ap=mt[:, :nib].rearrange(
                                        "p (g e) -> p g e", e=P
                                    ),
                                    in_ap=table,
                                    idxs_ap=it[:, : nib // 16],
                                    num_idxs=nib,
                                    num_idxs_reg=nib,
                                    elem_size=P,
                                    single_packet=cfg.single_packet,
                                    queue_num=qctr[0] % cfg.nq,
                                )
                                qctr[0] += 1
                            else:
                                e0 = 0
                                for j in range(j0, j0 + cfg.scw):
                                    ne = int(G[j, b]) * P
                                    if ne == 0:
                                        continue
                                    nc.gpsimd.dma_gather(
                                        out_ap=mt[:, e0 : e0 + ne].rearrange(
                                            "p (g e) -> p g e", e=P
                                        ),
                                        in_ap=table,
                                        idxs_ap=it[:, e0 // 16 : (e0 + ne) // 16],
                                        num_idxs=ne,
                                        num_idxs_reg=ne,
                                        elem_size=P,
                                        single_packet=cfg.single_packet,
                                        queue_num=qctr[0] % cfg.nq,
                                    )
                                    qctr[0] += 1
                                    e0 += ne
                        mg.append(mt)

                    bpos = [0] * nbuck
                    for jl in range(cfg.scw):
                        j = j0 + jl
                        gw = int(G[j].sum())
                        lc = rdcol - sc_rd0
                        if cfg.batch_st and gw:
                            stw = p_s.tile([P, max(P, gw * P)], bf16, tag="sel")
                            in0 = (
                                rdt[:, lc : lc + gw]
                                .unsqueeze(2)
                                .broadcast_to([P, gw, P])
                            )
                            in1 = iota_t[:].unsqueeze(1).broadcast_to([P, gw, P])
                            nc.vector.tensor_tensor(
                                out=stw[:, : gw * P].rearrange("p (g d) -> p g d", d=P),
                                in0=in0,
                                in1=in1,
                                op=EQ,
                            )
                        ps = p_ps.tile([P, P], f32, space="PSUM", tag="agg")
                        k = 0
                        for b in range(nbuck):
                            for _g in range(int(G[j, b])):
                                if cfg.batch_st:
                                    stk = stw[:, k * P : (k + 1) * P]
                                else:
                                    st = p_s.tile([P, P], bf16, tag="sel")
                                    nc.vector.tensor_scalar(
                                        out=st[:],
                                        in0=iota_t[:],
                                        scalar1=rdt[:, lc + k : lc + k + 1],
                                        scalar2=None,
                                        op0=EQ,
                                    )
                                    stk = st[:]
                                e0 = bpos[b]
                                if layer1:
                                    nc.tensor.matmul(
                                        ps[:],
                                        lhsT=mg[b][:, e0 : e0 + P],
                                        rhs=stk,
                                        start=(k == 0),
                                        stop=(k == gw - 1),
                                    )
                                else:
                                    nc.tensor.matmul(
                                        ps[:],
                                        lhsT=stk,
                                        rhs=mg[b][:, e0 : e0 + P],
                                        start=(k == 0),
                                        stop=(k == gw - 1),
                                    )
                                bpos[b] += P
                                rdcol += 1
                                k += 1
                        assert gw > 0, (s, j)
                        if layer1:
                            c1 = p_eo.tile([P, P], bf16, tag="c1")
                            if cfg.dense_self:
                                nc.vector.tensor_tensor(
                                    out=c1[:],
                                    in0=ps[:],
                                    in1=dn[:, jl * P : (jl + 1) * P],
                                    op=ADD,
                                )
                            else:
                                nc.scalar.activation(c1[:], ps[:], ACT_COPY)
                            ps2 = p_ps2.tile([P, P], f32, space="PSUM", tag="t1")
                            nc.tensor.matmul(
                                ps2[:], lhsT=w1_t[:], rhs=c1[:], start=True, stop=True
                            )
                            rt = p_eo.tile([P, P], bf16, tag="rt")
                            nc.scalar.activation(rt[:], ps2[:], ACT_RELU)
                            ps3 = p_ps.tile([P, P], f32, space="PSUM", tag="mm2")
                            nc.tensor.matmul(
                                ps3[:], lhsT=rt[:], rhs=w2_t[:], start=True, stop=True
                            )
                            t2 = p_eo.tile([P, P], bf16, tag="t2")
                            nc.scalar.activation(
                                t2[:], ps3[:], ACT_COPY, scale=dinv2_own[:, j : j + 1]
                            )
                            if cfg.chunk_ag:
                                kk = chunk_of_w[j]
                                nc.sync.dma_start(
                                    ts2own[kk]
                                    .ap()
                                    .rearrange("(t p) f -> p t f", p=P)[
                                        :, row_of_w[j], :
                                    ],
                                    t2[:],
                                )
                                if j in chunk_ends and not no_collective:
                                    nc.gpsimd.collective_compute(
                                        "AllGather",
                                        mybir.AluOpType.bypass,
                                        replica_groups=[list(range(cfg.n_cores))],
                                        ins=[ts2own[kk][:, :]],
                                        outs=[ts2f[kk][:, :]],
                                    )
                            else:
                                nc.sync.dma_start(ts2s_r[:, j, :], t2[:])
                        else:
                            if cfg.dense_self:
                                o1 = p_eo.tile([P, P], f32, tag="o1")
                                nc.vector.tensor_tensor(
                                    out=o1[:],
                                    in0=ps[:],
                                    in1=dn[:, jl * P : (jl + 1) * P],
                                    op=ADD,
                                )
                                o2 = p_eo.tile([P, P], f32, tag="o2")
                                nc.scalar.activation(
                                    o2[:], o1[:], ACT_COPY, scale=dinv_own[:, j : j + 1]
                                )
                            else:
                                o2 = p_eo.tile([P, P], f32, tag="o2")
                                nc.scalar.activation(
                                    o2[:], ps[:], ACT_COPY, scale=dinv_own[:, j : j + 1]
                                )
                            o3 = p_eo.tile([P, P], out_dt, tag="o3")
                            nc.vector.tensor_tensor(
                                out=o3[:], in0=o2[:], in1=b2_t[:], op=ADD
                            )
                            nc.sync.dma_start(out_r[:, j, :], o3[:])

        for rep in range(reps):
            aggregate(layer1=True, rep=rep)
            if not cfg.chunk_ag and not no_collective:
                nc.gpsimd.collective_compute(
                    "AllGather",
                    mybir.AluOpType.bypass,
                    replica_groups=[list(range(cfg.n_cores))],
                    ins=[ts2s[:, :]],
                    outs=[ts2f0[:, :]],
                )
            aggregate(layer1=False, rep=rep)

        cpool.__exit__(None, None, None)

    nc.compile()
    return nc


# ----------------------------------------------------------------------------
# Entry point
# ----------------------------------------------------------------------------

_CACHE = {}


def _prep_inputs(cfg: Cfg, pre, x, W1, W2, b2):
    n = cfg.n
    dinv = 1.0 / np.sqrt(pre["degp"])
    xsp = np.zeros((cfg.npad, P), dtype=np.float32)
    xsp[pre["pad_id"](np.arange(n))] = np.asarray(x, np.float32)
    xsp *= dinv[:, None]
    xs = xsp.astype(ml_dtypes.bfloat16)

    iota = np.broadcast_to(np.arange(P, dtype=np.float32), (P, P)).astype(
        ml_dtypes.bfloat16
    )
    in_maps = []
    for c in range(cfg.n_cores):
        m = {
            "xs": xs,
            "w1": np.asarray(W1, np.float32).astype(ml_dtypes.bfloat16),
            "w2": np.asarray(W2, np.float32).astype(ml_dtypes.bfloat16),
            "b2r": np.broadcast_to(np.asarray(b2, np.float32), (P, P)).copy(),
            "iota": np.ascontiguousarray(iota),
            "deg_own": pre["deg_all"][:, c * cfg.wpc : (c + 1) * cfg.wpc],
            "idx1": pre["s1"]["idx_all"][c],
            "rd1": np.ascontiguousarray(pre["s1"]["rd_all"][c]),
        }
        if cfg.dense_self:
            m["xso"] = np.ascontiguousarray(xs[c * cfg.ppc : (c + 1) * cfg.ppc])
        if cfg.chunk_ag:
            m["idx2"] = pre["s2"]["idx_all"][c]
            m["rd2"] = np.ascontiguousarray(pre["s2"]["rd_all"][c])
        in_maps.append(m)
    return in_maps


def _sched_key(s):
    return (s["gtot"], s["icols"], s["G"].tobytes(), s["ni"].tobytes())


def _get_nc(cfg: Cfg, pre):
    key = (cfg, _sched_key(pre["s1"]), _sched_key(pre["s2"]))
    if key not in _CACHE:
        _CACHE[key] = _build(cfg, pre["s1"], pre["s2"])
    return _CACHE[key]


def _kernel_impl(cfg: Cfg, x, edge_index, W1, b1, W2, b2):
    from concourse.bass_utils import run_bass_kernel_spmd

    assert np.allclose(b1, 0.0), "kernel assumes b1 == 0 (spec fill: zeros)"

    pre = _preprocess(cfg, np.asarray(edge_index, dtype=np.int64))
    nc = _get_nc(cfg, pre)
    in_maps = _prep_inputs(cfg, pre, x, W1, W2, b2)

    res = run_bass_kernel_spmd(nc, in_maps, list(range(cfg.n_cores)))
    parts = [
        res.results[c]["out"][: cfg.npc].astype(np.float32)
        for c in range(cfg.n_cores)
    ]
    return np.concatenate(parts, axis=0)


def kernel(x, edge_index, W1, b1, W2, b2):
    return _kernel_impl(make_cfg(), x, edge_index, W1, b1, W2, b2)


# ----------------------------------------------------------------------------
# Timing support
# ----------------------------------------------------------------------------


def _make_runner(nc, n_cores):
    import jax
    from jax.sharding import Mesh, NamedSharding, PartitionSpec
    from jax.experimental.shard_map import shard_map

    from concourse import bass2jax, mybir

    bass2jax.install_neuronx_cc_hook()
    partition_name = nc.partition_id_tensor.name if nc.partition_id_tensor else None
    in_names, out_names, out_avals, zero_outs = [], [], [], []
    for alloc in nc.m.functions[0].allocations:
        if not isinstance(alloc, mybir.MemoryLocationSet):
            continue
        name = alloc.memorylocations[0].name
        if alloc.kind == "ExternalInput":
            if name != partition_name:
                in_names.append(name)
        elif alloc.kind == "ExternalOutput":
            shape = tuple(alloc.tensor_shape)
            dtype = mybir.dt.np(alloc.dtype)
            out_names.append(name)
            out_avals.append(jax.core.ShapedArray(shape, dtype))
            zero_outs.append(np.zeros(shape, dtype))
    n_params = len(in_names)
    all_in_names = list(in_names) + list(out_names)
    if partition_name is not None:
        all_in_names.append(partition_name)

    def _body(*args):
        operands = list(args)
        if partition_name is not None:
            operands.append(bass2jax.partition_id_tensor())
        outs = bass2jax._bass_exec_p.bind(
            *operands,
            out_avals=tuple(out_avals),
            in_names=tuple(all_in_names),
            out_names=tuple(out_names),
            lowering_input_output_aliases=(),
            sim_require_finite=True,
            sim_require_nnan=True,
            nc=nc,
        )
        return tuple(outs)

    devices = jax.devices()[:n_cores]
    mesh = Mesh(np.asarray(devices), ("core",))
    in_specs = (PartitionSpec("core"),) * (n_params + len(out_names))
    out_specs = (PartitionSpec("core"),) * len(out_names)
    fn = jax.jit(
        shard_map(
            _body, mesh=mesh, in_specs=in_specs, out_specs=out_specs, check_rep=False
        ),
        keep_unused=True,
    )
    sharding = NamedSharding(mesh, PartitionSpec("core"))

    def run(in_maps, iters=1):
        import time as _t

        concat = [
            np.concatenate([np.asarray(in_maps[c][n]) for c in range(n_cores)], axis=0)
            for n in in_names
        ]
        concat += [
            np.zeros((n_cores * z.shape[0], *z.shape[1:]), z.dtype) for z in zero_outs
        ]
        dev_in = [jax.device_put(a, sharding) for a in concat]
        outs = fn(*dev_in)
        jax.block_until_ready(outs)
        times = []
        for _ in range(iters):
            t0 = _t.perf_counter()
            outs = fn(*dev_in)
            jax.block_until_ready(outs)
            times.append(_t.perf_counter() - t0)
        return outs, out_names, out_avals, times

    return run


def time_kernel(x, edge_index, W1, b1, W2, b2, iters=30, reps=9):
    cfg = make_cfg()
    pre = _preprocess(cfg, np.asarray(edge_index, dtype=np.int64))
    in_maps = _prep_inputs(cfg, pre, x, W1, W2, b2)

    nc1 = _get_nc(cfg, pre)
    run1 = _make_runner(nc1, cfg.n_cores)
    _, _, _, t1 = run1(in_maps, iters=iters)

    ncR = _build(cfg, pre["s1"], pre["s2"], reps=reps)
    runR = _make_runner(ncR, cfg.n_cores)
    _, _, _, tR = runR(in_maps, iters=iters)

    est = (min(tR) - min(t1)) / (reps - 1)
    m1 = sorted(t1)[len(t1) // 2]
    mR = sorted(tR)[len(tR) // 2]
    print(
        f"(x1: min {min(t1)*1e3:.3f} med {m1*1e3:.3f} ms; "
        f"x{reps}: min {min(tR)*1e3:.3f} med {mR*1e3:.3f} ms; "
        f"med-slope {(mR-m1)/(reps-1)*1e6:.0f} us)"
    )
    return est * 1e9
